# revision 6
# baseline (speedup 1.0000x reference)
"""GIN GNN kernel for 8 Trainium2 NeuronCores.

Sharding: nodes split 5000/core (5120 slots incl. 120 pad). Edges sharded by
destination owner. Aggregation: host packs edges into (tile, lane, round)
slots; dma_gather fetches source rows node-major; PE matmul with identity rhs
transpose-accumulates each 128-edge chunk into PSUM [128f x 128dst].
int16 gather indices -> two static 32768-row windows into X.
MLP/BN/pooling stay feature-major; BN stats + graph pooling all-reduced.
"""
import sys

sys.path.insert(0, '/opt/trn_rl_repo')

import numpy as np
import concourse.bass as bass  # noqa: F401
import concourse.mybir as mybir
import concourse.tile as tile
from concourse import bacc
from concourse.bass_utils import run_bass_kernel_spmd

P = 128
F32 = mybir.dt.float32
BF16 = mybir.dt.bfloat16
I16 = mybir.dt.int16


class Cfg:
    N_NODES = 40000
    N_EDGES = 640000
    SD = 128
    HL = 2
    N_GRAPHS = 256
    VOCAB = 1340
    N_CLASS = 41
    BN_EPS = 1e-5
    NCORES = 8
    WINDOW = 32768
    SG_TILES = 2          # dst tiles per gather supergroup
    GATHER_DT = 'bf16'    # 'f32' or 'bf16' for the X gather path
    MAX_GIN = 4           # debug: truncate GIN layers
    SKIP_POOL = False     # debug: skip pooling + FC head

    def __init__(self, **kw):
        for k, v in kw.items():
            setattr(self, k, v)
        assert self.N_NODES % self.NCORES == 0
        self.REAL_PC = self.N_NODES // self.NCORES          # real nodes per core
        self.TILES_PC = -(-self.REAL_PC // P)               # dst tiles per core
        self.SLOTS_PC = self.TILES_PC * P                   # slots per core
        self.SLOTS = self.SLOTS_PC * self.NCORES
        self.B_BASE = self.SLOTS - self.WINDOW              # hi window base
        assert self.B_BASE >= 0 and self.REAL_PC < self.WINDOW
        self.EMB_ROWS = -(-(self.VOCAB + 1) // P) * P       # emb padded rows
        assert self.EMB_ROWS <= self.WINDOW
        # dummy (zero) slots reachable from each window
        self.DUMMY_LO = self.REAL_PC                        # core 0 first pad slot
        self.DUMMY_HI = (self.NCORES - 1) * self.SLOTS_PC + self.REAL_PC
        assert self.DUMMY_LO < self.WINDOW
        assert self.B_BASE <= self.DUMMY_HI < self.B_BASE + self.WINDOW
        self.MLP_GRP = 512
        self.NGRP = -(-self.SLOTS_PC // self.MLP_GRP)


def node_slot(cfg, n):
    """Global slot id of original node id n (vectorized)."""
    core = n // cfg.REAL_PC
    return core * cfg.SLOTS_PC + n % cfg.REAL_PC


def wrap_idx(vals):
    """Pack index list into [16, ceil(len/16)] int16 (i -> [i%16, i//16])."""
    n = len(vals)
    ncols = -(-n // 16)
    out = np.zeros((16, ncols), dtype=np.int16)
    pad = np.full(ncols * 16 - n, 0, dtype=np.int64)
    v = np.concatenate([np.asarray(vals, dtype=np.int64), pad])
    out[:, :] = v.reshape(ncols, 16).T
    return out


def build_schedule(cfg, edge_index, x_tokens):
    """Host-side edge packing. Returns uniform per-tile round counts and
    per-core int16 index planes for the layer-1 (emb) and layer-2+ gathers."""
    src = np.asarray(edge_index[0], dtype=np.int64)
    dst = np.asarray(edge_index[1], dtype=np.int64)
    toks = np.asarray(x_tokens, dtype=np.int64)
    ssl = node_slot(cfg, src)                   # source slot per edge
    core = dst // cfg.REAL_PC
    rem = dst % cfg.REAL_PC
    tl = rem // P                               # tile within core
    lane = rem % P

    order = np.lexsort((ssl, lane, tl, core))
    src_s, ssl_s, core_s, tl_s, lane_s = (
        src[order], ssl[order], core[order], tl[order], lane[order])

    NC, T = cfg.NCORES, cfg.TILES_PC
    # per-(core,tile,lane) counts
    key = (core_s * T + tl_s) * P + lane_s
    nAonly = np.bincount(key[ssl_s < cfg.B_BASE], minlength=NC * T * P)
    nBonly = np.bincount(key[ssl_s >= cfg.WINDOW], minlength=NC * T * P)
    deg = np.bincount(key, minlength=NC * T * P)
    nAonly = nAonly.reshape(NC, T, P)
    nBonly = nBonly.reshape(NC, T, P)
    deg = deg.reshape(NC, T, P)

    RA = np.maximum(1, nAonly.max(axis=(0, 2)))            # per-tile, >=1
    RB0 = nBonly.max(axis=(0, 2))
    D = deg.max(axis=(0, 2))
    RB = np.maximum(RB0, D - RA)

    # per-lane A-count: a = min(nAonly+nflex, RA); rest -> B
    nflex = deg - nAonly - nBonly
    acnt = np.minimum(nAonly + nflex, RA[None, :, None])

    # slot-fill: for each (core,tile): A block RA[t]*128 slots, B block RB[t]*128
    # slot (r, lane) = lane's r-th edge (sorted by src slot asc)
    starts = np.zeros(NC * T * P + 1, dtype=np.int64)
    np.cumsum(deg.reshape(-1), out=starts[1:])

    idx_l2 = []   # per core: [16*8, cols] int16
    idx_l1 = []
    sgs = []      # emission schedule (shared): list of supergroup dicts
    # column cursor in units of idx columns (16 idx each)
    cur = 0
    tiles = list(range(T))
    sg_list = [tiles[i:i + cfg.SG_TILES] for i in range(0, T, cfg.SG_TILES)]
    for sg in sg_list:
        nA = int(sum(RA[t] for t in sg))
        nB = int(sum(RB[t] for t in sg))
        sgs.append({'tiles': sg, 'nA': nA, 'nB': nB,
                    'colA': cur, 'colB': cur + nA * 8,
                    'RA': [int(RA[t]) for t in sg],
                    'RB': [int(RB[t]) for t in sg]})
        cur += (nA + nB) * 8
    total_cols = cur

    for c in range(NC):
        valsA2 = np.full((T, int(RA.max()), P), cfg.DUMMY_LO, dtype=np.int64)
        valsB2 = np.full((T, int(RB.max()) if RB.max() > 0 else 1, P),
                         cfg.DUMMY_HI - cfg.B_BASE, dtype=np.int64)
        valsA1 = np.full_like(valsA2, cfg.VOCAB)
        valsB1 = np.full_like(valsB2, cfg.VOCAB)
        for t in range(T):
            for l in range(P):
                k = (c * T + t) * P + l
                d = deg[c, t, l]
                if d == 0:
                    continue
                e0 = starts[k]
                a = int(acnt[c, t, l])
                s_sl = ssl_s[e0:e0 + d]
                s_or = src_s[e0:e0 + d]
                valsA2[t, :a, l] = s_sl[:a]
                valsA1[t, :a, l] = toks[s_or[:a]]
                b = d - a
                if b:
                    valsB2[t, :b, l] = s_sl[a:] - cfg.B_BASE
                    valsB1[t, :b, l] = toks[s_or[a:]]
        # flatten into the column layout
        cols2 = np.zeros((16, total_cols), dtype=np.int16)
        cols1 = np.zeros((16, total_cols), dtype=np.int16)
        for sgd in sgs:
            for arr2, arr1, key_n, key_c, Rs in (
                    (valsA2, valsA1, 'nA', 'colA', sgd['RA']),
                    (valsB2, valsB1, 'nB', 'colB', sgd['RB'])):
                cc = sgd[key_c]
                for t, R in zip(sgd['tiles'], Rs):
                    for r in range(R):
                        v2 = arr2[t, r, :]
                        v1 = arr1[t, r, :]
                        cols2[:, cc:cc + 8] = v2.reshape(8, 16).T
                        cols1[:, cc:cc + 8] = v1.reshape(8, 16).T
                        cc += 8
        idx_l2.append(np.tile(cols2, (8, 1)))
        idx_l1.append(np.tile(cols1, (8, 1)))

    # x0 / local-token gather indices + batch-local plane are built in kernel()
    return {'sgs': sgs, 'total_cols': total_cols,
            'idx_l1': idx_l1, 'idx_l2': idx_l2,
            'RA': RA, 'RB': RB}


def emit_kernel(nc, tc, cfg, sched, names):
    """Emit the full 4-layer GIN + pooling + FC program (identical all cores)."""
    dtg = BF16 if cfg.GATHER_DT == 'bf16' else F32
    T, S = cfg.TILES_PC, cfg.SLOTS_PC
    Cmax = max(sg['nA'] + sg['nB'] for sg in sched['sgs'])

    d = {k: nc.dram_tensor(k, list(v.shape), mybir.dt.from_np(v.dtype),
                           kind='ExternalInput')
         for k, v in names.items()}
    out_d = nc.dram_tensor('out', [cfg.N_GRAPHS, cfg.N_CLASS], F32,
                           kind='ExternalOutput')

    emb_g = nc.dram_tensor('emb_g', [cfg.EMB_ROWS, P], dtg, kind='Internal')
    xnm = nc.dram_tensor('xnm', [cfg.SLOTS, P], dtg, kind='Internal',
                         addr_space='Shared')
    xstage = nc.dram_tensor('xstage', [S, P], dtg, kind='Internal')
    bn_in = nc.dram_tensor('bn_in', [P, 2], F32, kind='Internal')
    bn_out = nc.dram_tensor('bn_out', [P, 2], F32, kind='Internal',
                            addr_space='Shared')
    pool_in = nc.dram_tensor('pool_in', [P, cfg.N_GRAPHS], F32, kind='Internal')
    pool_out = nc.dram_tensor('pool_out', [P, cfg.N_GRAPHS], F32,
                              kind='Internal', addr_space='Shared')

    RG = [list(range(cfg.NCORES))]

    with tc.tile_pool(name='pp', bufs=1) as PP, \
         tc.tile_pool(name='rot', bufs=2) as RP, \
         tc.tile_pool(name='psA', bufs=2, space='PSUM') as PSA, \
         tc.tile_pool(name='psB', bufs=2, space='PSUM') as PSB, \
         tc.tile_pool(name='psC', bufs=2, space='PSUM') as PSC:

        def sb(name, shape, dt=F32):
            t_ = PP.tile(shape, dt, tag=name)
            return t_

        def load(name, shape=None, dt=F32):
            t_ = sb(name, shape or list(names[name].shape), dt)
            nc.sync.dma_start(t_[:], d[name][:])
            return t_

        # ---- constants / params to SBUF
        idx1 = load('idx1', dt=I16)
        idx2 = load('idx2', dt=I16)
        idxx0 = load('idxx0', dt=I16)
        batchl = load('batchl')
        iden = load('iden')                       # [128,128] f32 identity
        ideng = sb('ideng', [P, P], dtg)
        nc.vector.tensor_copy(ideng[:], iden[:])
        iotag = load('iotag')                     # [128, G] f32 iota
        zerov = sb('zerov', [P, 1])
        nc.vector.memset(zerov[:], 0.0)
        Ws = {k: load(k) for k in
              ('Win1', 'Win2', 'Wh10', 'Wh20', 'Wh11', 'Wh21',
               'Wout1', 'Wout2', 'Wfc1', 'Wfc2')}
        Bs = {k: load(k, [P, 1]) for k in
              ('bin1', 'bin2', 'bh10', 'bh20', 'bh11', 'bh21',
               'bout1', 'bout2', 'bfc1', 'g_in', 'b_in',
               'g_h0', 'b_h0', 'g_h1', 'b_h1')}
        bfc2 = load('bfc2', [cfg.N_CLASS, 1])

        # ---- emb: pad + cast to gather dtype in DRAM
        ET = cfg.EMB_ROWS // P
        embsb = sb('embsb', [P, ET, P])
        nc.sync.dma_start(
            embsb[:], d['embp'][:].rearrange('(t p) f -> p t f', p=P))
        embg_sb = sb('embg_sb', [P, ET, P], dtg)
        nc.vector.tensor_copy(embg_sb[:], embsb[:])
        nc.sync.dma_start(emb_g[:].rearrange('(t p) f -> p t f', p=P),
                          embg_sb[:])

        # ---- X0^T local: gather emb rows for local slots, transpose per tile
        XT = sb('XT', [P, S])
        HT = sb('HT', [P, S])
        ZT = sb('ZT', [P, S])
        x0nm = sb('nm32', [P, T, P])
        nc.gpsimd.dma_gather(x0nm[:], d['embp'][:], idxx0[:], S, S, P,
                             single_packet=False)
        for t in range(T):
            ps = PSC.tile([P, 256], F32, tag='tr')
            nc.tensor.matmul(ps[:, :P], lhsT=x0nm[:, t, :], rhs=iden[:],
                             start=True, stop=True)
            nc.vector.tensor_copy(XT[:, t * P:(t + 1) * P], ps[:, :P])

        layers = [
            ('Win1', 'bin1', 'Win2', 'bin2', 'g_in', 'b_in'),
            ('Wh10', 'bh10', 'Wh20', 'bh20', 'g_h0', 'b_h0'),
            ('Wh11', 'bh11', 'Wh21', 'bh21', 'g_h1', 'b_h1'),
            ('Wout1', 'bout1', 'Wout2', 'bout2', None, None),
        ]

        layers = layers[:cfg.MAX_GIN]
        for li, (w1, b1, w2, b2, ga, be) in enumerate(layers):
            lay1 = li == 0
            idxt = idx1 if lay1 else idx2
            # ---- aggregation
            for sgd in sched['sgs']:
                nA, nB = sgd['nA'], sgd['nB']
                stage = RP.tile([P, Cmax, P], dtg, tag='stage')
                if lay1:
                    srcA = emb_g[:]
                    srcB = emb_g[:]
                else:
                    srcA = xnm[0:cfg.WINDOW, :]
                    srcB = xnm[cfg.B_BASE:cfg.B_BASE + cfg.WINDOW, :]
                if nA:
                    nc.gpsimd.dma_gather(
                        stage[:, 0:nA, :], srcA,
                        idxt[:, sgd['colA']:sgd['colA'] + nA * 8],
                        nA * P, nA * P, P, single_packet=False)
                if nB:
                    nc.gpsimd.dma_gather(
                        stage[:, nA:nA + nB, :], srcB,
                        idxt[:, sgd['colB']:sgd['colB'] + nB * 8],
                        nB * P, nB * P, P, single_packet=False)
                offA = 0
                offB = nA
                for ti, t in enumerate(sgd['tiles']):
                    ra, rb = sgd['RA'][ti], sgd['RB'][ti]
                    chunks = [offA + r for r in range(ra)] + \
                             [offB + r for r in range(rb)]
                    offA += ra
                    offB += rb
                    ps = PSA.tile([P, P], F32, tag='agg')
                    for i, cch in enumerate(chunks):
                        nc.tensor.matmul(ps[:], lhsT=stage[:, cch, :],
                                         rhs=ideng[:], start=(i == 0),
                                         stop=(i == len(chunks) - 1))
                    c0 = t * P
                    nc.vector.tensor_add(out=HT[:, c0:c0 + P],
                                         in0=XT[:, c0:c0 + P], in1=ps[:])
            # ---- MLP: ZT = relu(relu(HT.T@W1+b1)@W2+b2) (feature-major)
            for g in range(cfg.NGRP):
                c0 = g * cfg.MLP_GRP
                c1 = min(S, c0 + cfg.MLP_GRP)
                w = c1 - c0
                pm1 = PSB.tile([P, cfg.MLP_GRP], F32, tag='mlp')
                nc.tensor.matmul(pm1[:, :w], lhsT=Ws[w1][:], rhs=HT[:, c0:c1],
                                 start=True, stop=True)
                m1 = RP.tile([P, cfg.MLP_GRP], F32, tag='m1')
                nc.scalar.activation(m1[:, :w], pm1[:, :w],
                                     mybir.ActivationFunctionType.Relu,
                                     bias=Bs[b1][:])
                pm2 = PSB.tile([P, cfg.MLP_GRP], F32, tag='mlp')
                nc.tensor.matmul(pm2[:, :w], lhsT=Ws[w2][:], rhs=m1[:, :w],
                                 start=True, stop=True)
                nc.scalar.activation(ZT[:, c0:c1], pm2[:, :w],
                                     mybir.ActivationFunctionType.Relu,
                                     bias=Bs[b2][:])
            if cfg.REAL_PC < S:
                nc.vector.memset(ZT[:, cfg.REAL_PC:S], 0.0)
            # ---- BN (layers 0..2)
            if ga is not None:
                s1p = sb('s1p', [P, cfg.NGRP])
                s2p = sb('s2p', [P, cfg.NGRP])
                for g in range(cfg.NGRP):
                    c0 = g * cfg.MLP_GRP
                    c1 = min(S, c0 + cfg.MLP_GRP)
                    sq = RP.tile([P, cfg.MLP_GRP], F32, tag='sq')
                    nc.scalar.activation(sq[:, :c1 - c0], ZT[:, c0:c1],
                                         mybir.ActivationFunctionType.Square,
                                         bias=zerov[:])
                    nc.vector.tensor_reduce(
                        out=s1p[:, g:g + 1], in_=ZT[:, c0:c1],
                        axis=mybir.AxisListType.X, op=mybir.AluOpType.add)
                    nc.vector.tensor_reduce(
                        out=s2p[:, g:g + 1], in_=sq[:, :c1 - c0],
                        axis=mybir.AxisListType.X, op=mybir.AluOpType.add)
                bstat = sb('bstat', [P, 2])
                nc.vector.tensor_reduce(out=bstat[:, 0:1], in_=s1p[:],
                                        axis=mybir.AxisListType.X,
                                        op=mybir.AluOpType.add)
                nc.vector.tensor_reduce(out=bstat[:, 1:2], in_=s2p[:],
                                        axis=mybir.AxisListType.X,
                                        op=mybir.AluOpType.add)
                nc.sync.dma_start(bn_in[:], bstat[:])
                nc.gpsimd.collective_compute(
                    'AllReduce', mybir.AluOpType.add, replica_groups=RG,
                    ins=[bn_in[:]], outs=[bn_out[:]])
                bagg = sb('bagg', [P, 2])
                nc.sync.dma_start(bagg[:], bn_out[:])
                mu = sb('mu', [P, 1])
                ex2 = sb('ex2', [P, 1])
                inv_n = 1.0 / cfg.N_NODES
                nc.scalar.mul(mu[:], bagg[:, 0:1], inv_n)
                nc.scalar.mul(ex2[:], bagg[:, 1:2], inv_n)
                var = sb('var', [P, 1])
                nc.vector.tensor_tensor(out=var[:], in0=mu[:], in1=mu[:],
                                        op=mybir.AluOpType.mult)
                nc.vector.tensor_tensor(out=var[:], in0=ex2[:], in1=var[:],
                                        op=mybir.AluOpType.subtract)
                nc.vector.tensor_scalar_add(var[:], var[:],
                                            float(cfg.BN_EPS))
                sd = sb('sd', [P, 1])
                nc.scalar.activation(sd[:], var[:],
                                     mybir.ActivationFunctionType.Sqrt,
                                     bias=zerov[:])
                rstd = sb('rstd', [P, 1])
                nc.vector.reciprocal(rstd[:], sd[:])
                scal = sb('scal', [P, 1])
                nc.vector.tensor_tensor(out=scal[:], in0=Bs[ga][:],
                                        in1=rstd[:], op=mybir.AluOpType.mult)
                shif = sb('shif', [P, 1])
                nc.vector.tensor_tensor(out=shif[:], in0=mu[:], in1=scal[:],
                                        op=mybir.AluOpType.mult)
                nc.vector.tensor_tensor(out=shif[:], in0=Bs[be][:],
                                        in1=shif[:],
                                        op=mybir.AluOpType.subtract)
                for g in range(cfg.NGRP):
                    c0 = g * cfg.MLP_GRP
                    c1 = min(S, c0 + cfg.MLP_GRP)
                    nc.scalar.activation(ZT[:, c0:c1], ZT[:, c0:c1],
                                         mybir.ActivationFunctionType.Identity,
                                         bias=shif[:], scale=scal[:])
                if cfg.REAL_PC < S:
                    nc.vector.memset(ZT[:, cfg.REAL_PC:S], 0.0)
                # ---- write node-major + AllGather for next layer's gathers
                xnms = sb('xnms', [P, T, P], dtg)
                for t in range(T):
                    ps = PSC.tile([P, 256], F32, tag='tr')
                    nc.tensor.matmul(ps[:, :P], lhsT=ZT[:, t * P:(t + 1) * P],
                                     rhs=iden[:], start=True, stop=True)
                    nc.vector.tensor_copy(xnms[:, t, :], ps[:, :P])
                nc.sync.dma_start(
                    xstage[:].rearrange('(t p) f -> p t f', p=P), xnms[:])
                nc.gpsimd.collective_compute(
                    'AllGather', mybir.AluOpType.bypass, replica_groups=RG,
                    ins=[xstage[:]], outs=[xnm[:]])
            # rotate buffers: next layer input = ZT
            XT, ZT = ZT, XT

        if cfg.SKIP_POOL:
            PG0 = min(P, cfg.N_GRAPHS)
            nt0 = -(-cfg.N_GRAPHS // PG0)
            osb0 = sb('osb0', [PG0, nt0, cfg.N_CLASS])
            nc.vector.memset(osb0[:], 0.0)
            nc.sync.dma_start(
                out_d[:].rearrange('(j p) c -> p j c', p=PG0), osb0[:])
            return out_d
        X4 = XT  # after swap, layer-4 output lives here
        # ---- pooling: P^T[f,g] = sum_t x4nm_t.T @ onehot_t
        x4nm = sb('nm32', [P, T, P])
        for t in range(T):
            ps = PSC.tile([P, 256], F32, tag='tr')
            nc.tensor.matmul(ps[:, :P], lhsT=X4[:, t * P:(t + 1) * P],
                             rhs=iden[:], start=True, stop=True)
            nc.vector.tensor_copy(x4nm[:, t, :], ps[:, :P])
        pps = PSB.tile([P, cfg.N_GRAPHS], F32, tag='pool')
        for t in range(T):
            oh = RP.tile([P, cfg.N_GRAPHS], F32, tag='oh')
            nc.vector.tensor_scalar(oh[:], iotag[:], batchl[:, t:t + 1], None,
                                    op0=mybir.AluOpType.is_equal)
            nc.tensor.matmul(pps[:], lhsT=x4nm[:, t, :], rhs=oh[:],
                             start=(t == 0), stop=(t == T - 1))
        psb_ = sb('psb', [P, cfg.N_GRAPHS])
        nc.vector.tensor_copy(psb_[:], pps[:])
        nc.sync.dma_start(pool_in[:], psb_[:])
        nc.gpsimd.collective_compute(
            'AllReduce', mybir.AluOpType.add, replica_groups=RG,
            ins=[pool_in[:]], outs=[pool_out[:]])
        pool_sb = sb('pool_sb', [P, cfg.N_GRAPHS])
        nc.sync.dma_start(pool_sb[:], pool_out[:])
        # ---- FC head (replicated)
        pf1 = PSB.tile([P, cfg.N_GRAPHS], F32, tag='pool')
        nc.tensor.matmul(pf1[:], lhsT=Ws['Wfc1'][:], rhs=pool_sb[:],
                         start=True, stop=True)
        f1 = sb('f1', [P, cfg.N_GRAPHS])
        nc.scalar.activation(f1[:], pf1[:],
                             mybir.ActivationFunctionType.Relu,
                             bias=Bs['bfc1'][:])
        pf2 = PSB.tile([P, cfg.N_GRAPHS], F32, tag='pool')
        nc.tensor.matmul(pf2[:cfg.N_CLASS, :], lhsT=Ws['Wfc2'][:], rhs=f1[:],
                         start=True, stop=True)
        ot = sb('ot', [cfg.N_CLASS, cfg.N_GRAPHS])
        nc.scalar.activation(ot[:], pf2[:cfg.N_CLASS, :],
                             mybir.ActivationFunctionType.Identity,
                             bias=bfc2[:])
        # transpose [41, G] -> [G, 41]
        PG = min(P, cfg.N_GRAPHS)
        ntr = -(-cfg.N_GRAPHS // PG)
        osb = sb('osb', [PG, ntr, cfg.N_CLASS])
        for j in range(ntr):
            g0 = j * PG
            g1 = min(cfg.N_GRAPHS, g0 + PG)
            ps = PSC.tile([P, 256], F32, tag='tr')
            nc.tensor.matmul(ps[:g1 - g0, :cfg.N_CLASS],
                             lhsT=ot[:, g0:g1],
                             rhs=iden[:cfg.N_CLASS, :cfg.N_CLASS],
                             start=True, stop=True)
            nc.vector.tensor_copy(osb[:g1 - g0, j, :],
                                  ps[:g1 - g0, :cfg.N_CLASS])
        nc.sync.dma_start(
            out_d[:].rearrange('(j p) c -> p j c', p=PG), osb[:])
    return out_d


def build_inputs(cfg, inputs, sched):
    """Per-core in_maps from full inputs."""
    toks = np.asarray(inputs['x_tokens'], dtype=np.int64)
    batch = np.asarray(inputs['batch'], dtype=np.int64)
    emb = np.asarray(inputs['emb'], dtype=np.float32)
    embp = np.zeros((cfg.EMB_ROWS, P), dtype=np.float32)
    embp[:cfg.VOCAB] = emb

    G = cfg.N_GRAPHS
    iotag = np.tile(np.arange(G, dtype=np.float32), (P, 1))
    iden = np.eye(P, dtype=np.float32)

    common = {
        'embp': embp, 'iden': iden, 'iotag': iotag,
        'Win1': np.asarray(inputs['Win1'], np.float32),
        'Win2': np.asarray(inputs['Win2'], np.float32),
        'Wh10': np.asarray(inputs['Wh1'][0], np.float32),
        'Wh20': np.asarray(inputs['Wh2'][0], np.float32),
        'Wh11': np.asarray(inputs['Wh1'][1], np.float32),
        'Wh21': np.asarray(inputs['Wh2'][1], np.float32),
        'Wout1': np.asarray(inputs['Wout1'], np.float32),
        'Wout2': np.asarray(inputs['Wout2'], np.float32),
        'Wfc1': np.asarray(inputs['Wfc1'], np.float32),
        'Wfc2': np.asarray(inputs['Wfc2'], np.float32),
        'bin1': np.asarray(inputs['bin1'], np.float32),
        'bin2': np.asarray(inputs['bin2'], np.float32),
        'bh10': np.asarray(inputs['bh1'][0], np.float32),
        'bh20': np.asarray(inputs['bh2'][0], np.float32),
        'bh11': np.asarray(inputs['bh1'][1], np.float32),
        'bh21': np.asarray(inputs['bh2'][1], np.float32),
        'bout1': np.asarray(inputs['bout1'], np.float32),
        'bout2': np.asarray(inputs['bout2'], np.float32),
        'bfc1': np.asarray(inputs['bfc1'], np.float32),
        'bfc2': np.asarray(inputs['bfc2'], np.float32),
        'g_in': np.asarray(inputs['g_in'], np.float32),
        'b_in': np.asarray(inputs['b_in'], np.float32),
        'g_h0': np.asarray(inputs['g_h'][0], np.float32),
        'b_h0': np.asarray(inputs['b_h'][0], np.float32),
        'g_h1': np.asarray(inputs['g_h'][1], np.float32),
        'b_h1': np.asarray(inputs['b_h'][1], np.float32),
    }

    in_maps = []
    for c in range(cfg.NCORES):
        n0 = c * cfg.REAL_PC
        # x0 gather: token of local slot, pads -> VOCAB (zero emb row)
        tk = np.full(cfg.SLOTS_PC, cfg.VOCAB, dtype=np.int64)
        tk[:cfg.REAL_PC] = toks[n0:n0 + cfg.REAL_PC]
        ix0 = np.tile(wrap_idx(tk), (8, 1))
        # batch-local [128, T]: graph of slot t*128+p, -1 for pads
        bl = np.full(cfg.SLOTS_PC, -1.0, dtype=np.float32)
        bl[:cfg.REAL_PC] = batch[n0:n0 + cfg.REAL_PC].astype(np.float32)
        bl = bl.reshape(cfg.TILES_PC, P).T.copy()
        m = dict(common)
        m['idx1'] = sched['idx_l1'][c]
        m['idx2'] = sched['idx_l2'][c]
        m['idxx0'] = ix0
        m['batchl'] = bl
        in_maps.append(m)
    return in_maps


_PROGRAM_CACHE = {}


def build_program(cfg, sched, in_maps):
    key = 'prog'
    if key in _PROGRAM_CACHE:
        return _PROGRAM_CACHE[key]
    nc = bacc.Bacc('TRN2', target_bir_lowering=False, debug=False,
                   num_devices=cfg.NCORES)
    names = {k: v for k, v in in_maps[0].items()}
    with tile.TileContext(nc) as tc:
        emit_kernel(nc, tc, cfg, sched, names)
    nc.compile()
    _PROGRAM_CACHE[key] = nc
    return nc


def run(cfg, inputs, trace=False):
    sched = build_schedule(cfg, np.asarray(inputs['edge_index']),
                           np.asarray(inputs['x_tokens']))
    in_maps = build_inputs(cfg, inputs, sched)
    nc = build_program(cfg, sched, in_maps)
    res = run_bass_kernel_spmd(nc, in_maps, list(range(cfg.NCORES)),
                               trace=trace)
    out = np.asarray(res.results[0]['out'], dtype=np.float32)
    return out, res


def kernel(**inputs):
    cfg = Cfg()
    out, _ = run(cfg, inputs, trace=False)
    return out
